# revision 35
# baseline (speedup 1.0000x reference)
"""Trainium2 Bass kernel for multi-head attention.

Problem: B=4, H=16, S=2048, D=128, fp32.
  scores = (q @ k^T) / sqrt(128); probs = softmax(scores, -1); out = probs @ v

Sharding: 64 (b,h) pairs -> 8 cores x 8 pairs. Fully independent per pair.

Layout (all-bf16 datapath; T-layout per (b,h) pair; s in halves of 1024):
  qT, kT: [D=128, S=2048] bf16 in SBUF. For each t-tile (128 keys):
    scoresT[t, s] = kT[:, t-tile].T @ qT   (PE, 2 matmuls N=512, bf16)
    expT = exp(scoresT / sqrt(D)) -> bf16  (ACT, fused scale, psum->sbuf)
    outT[d, s] += v_tile.T @ expT          (PE, 2 matmuls N=512, accum)
    eacc += expT                           (DVE bf16 adds, 2x mode)
  Denominator: eacc (tiles 0..14) collapsed over the 128 t-partitions by a
  PE ones-matmul pass started during exp15, tile 15 folded by a second
  accumulating ones-matmul pass; reciprocal (DVE approx, chunked),
  out = outT * rec (DVE), DMA out bf16 (host upcasts to fp32).

  The whole tail of half h (PV of tiles 14..15, sums, normalize, store) is
  EMITTED inside half h+1's t-loop after its second exp: the PE executes
  its queue in order, so this puts the next half's first QKs ahead of the
  exp15-gated tail matmuls -- ACT never idles at half boundaries (this
  was a ~1.5us stall per half when the tail was emitted inline). The PV
  consume lag is 2 tiles to match.

PSUM: 3 score slots x 2 banks (one doubles as the sums buffer during the
deferred tail) + outT accumulator 2 banks = 8 banks.

bf16 end-to-end error vs the fp32 reference is ~3.5e-3 relative
(threshold 2e-2).
"""

import sys

sys.path.insert(0, "/opt/trn_rl_repo")

import numpy as np

B, H, S, D = 4, 16, 2048, 128
N_CORES = 8
BH = B * H                      # 64 pairs
BH_PER_CORE = BH // N_CORES     # 8
T_TILES = S // 128              # 16
S_HALF = S // 2                 # 1024
SCALE = float(D) ** -0.5

_cache = {}


def _build_program():
    import concourse.tile as tile
    from concourse import bacc, mybir

    F32 = mybir.dt.float32
    BF16 = mybir.dt.bfloat16

    nc = bacc.Bacc("TRN2", target_bir_lowering=False, debug=False)

    qt = nc.dram_tensor("qt", [BH_PER_CORE, D, S], BF16, kind="ExternalInput")
    kt = nc.dram_tensor("kt", [BH_PER_CORE, D, S], BF16, kind="ExternalInput")
    # v pre-shuffled on host to [p, t, d] so the load is fully contiguous
    v = nc.dram_tensor("v", [BH_PER_CORE, 128, T_TILES * D], BF16, kind="ExternalInput")
    ot = nc.dram_tensor("ot", [BH_PER_CORE, D, S], BF16, kind="ExternalOutput")

    with tile.TileContext(nc) as tc:
        with (
            tc.tile_pool(name="const", bufs=1) as const,
            tc.tile_pool(name="rin", bufs=2) as rin,
            tc.tile_pool(name="exps", bufs=12) as exps,
            tc.tile_pool(name="accp", bufs=2) as accp,
            tc.tile_pool(name="outs", bufs=6) as outs,
            tc.tile_pool(name="psc", bufs=3, space="PSUM") as psc,
            tc.tile_pool(name="pacc", bufs=1, space="PSUM") as pacc,
        ):
            ones_f = const.tile([128, 128], F32)
            nc.vector.memset(ones_f[:], 1.0)
            ones_b = const.tile([128, 128], BF16)
            nc.vector.tensor_copy(ones_b[:], ones_f[:])

            pending_tail = [None]

            def flush_tail(final=False):
                if pending_tail[0] is not None:
                    t_fn = pending_tail[0]
                    pending_tail[0] = None
                    t_fn(final)

            for i in range(BH_PER_CORE):
                q_r = rin.tile([D, S], BF16, tag="q_r")
                k_r = rin.tile([D, S], BF16, tag="k_r")
                v_r = rin.tile([128, T_TILES, D], BF16, tag="v_r")
                # order: what the first tiles need comes first; v split so
                # the first PV tiles don't wait on one huge transfer
                nc.sync.dma_start(out=k_r[:, :128], in_=kt[i, :, :128])
                nc.sync.dma_start(out=q_r[:, :512], in_=qt[i, :, :512])
                vv = v[i].rearrange("p (t d) -> p t d", t=T_TILES)
                nc.sync.dma_start(out=v_r[:, 0:2], in_=vv[:, 0:2])
                nc.sync.dma_start(out=k_r[:, 128:512], in_=kt[i, :, 128:512])
                nc.sync.dma_start(out=q_r[:, 512:S_HALF], in_=qt[i, :, 512:S_HALF])
                nc.sync.dma_start(out=v_r[:, 2:4], in_=vv[:, 2:4])
                nc.sync.dma_start(out=k_r[:, 512:S_HALF], in_=kt[i, :, 512:S_HALF])
                nc.sync.dma_start(out=v_r[:, 4:8], in_=vv[:, 4:8])
                nc.sync.dma_start(out=v_r[:, 8:12], in_=vv[:, 8:12])
                nc.sync.dma_start(out=v_r[:, 12:16], in_=vv[:, 12:16])
                nc.sync.dma_start(out=q_r[:, S_HALF:], in_=qt[i, :, S_HALF:])
                nc.sync.dma_start(out=k_r[:, S_HALF:], in_=kt[i, :, S_HALF:])

                for h in range(2):
                    s0 = h * S_HALF
                    oacc = pacc.tile([128, S_HALF], F32, tag="oacc")
                    eacc = accp.tile([128, S_HALF], BF16, tag="eacc")

                    ets = [None] * T_TILES

                    def pv(t, oacc=oacc, v_r=v_r, ets=ets):
                        for c in range(0, S_HALF, 512):
                            nc.tensor.matmul(
                                oacc[:, c : c + 512],
                                v_r[:, t, :],
                                ets[t][:, c : c + 512],
                                start=(t == 0),
                                stop=(t == T_TILES - 1),
                            )

                    def consume(t, eacc=eacc, ets=ets, pv=pv):
                        pv(t)
                        if t == 1:
                            nc.vector.tensor_add(eacc[:], ets[0][:], ets[1][:])
                        elif 1 < t < T_TILES - 1:
                            nc.vector.tensor_add(eacc[:], eacc[:], ets[t][:])

                    for t in range(T_TILES):
                        sc = psc.tile([128, S_HALF], F32, tag="sc")
                        for c in range(0, S_HALF, 512):
                            nc.tensor.matmul(
                                sc[:, c : c + 512],
                                k_r[:, t * 128 : (t + 1) * 128],
                                q_r[:, s0 + c : s0 + c + 512],
                                start=True,
                                stop=True,
                            )
                        ets[t] = exps.tile(
                            [128, S_HALF], BF16, tag="et", name=f"et_{t}"
                        )
                        nc.scalar.activation(
                            ets[t][:],
                            sc[:],
                            mybir.ActivationFunctionType.Exp,
                            scale=SCALE,
                        )
                        if t == 1:
                            # previous half's tail lands here: after this
                            # half's first two QKs in PE program order
                            flush_tail()
                        if t >= 2:
                            consume(t - 2)
                    # PV14 + add14 run during exp15's window instead of in
                    # the deferred tail: 2 fewer matmuls ahead of the next
                    # half's 3rd QK.
                    consume(T_TILES - 2)

                    def make_tail(i=i, s0=s0, oacc=oacc, eacc=eacc,
                                  ets=ets, pv=pv, consume=consume):
                        def tail(final):
                            pv(T_TILES - 1)        # PV15
                            sacc = psc.tile(
                                [128, S_HALF], F32, tag="sc", name="sacc"
                            )
                            rec = outs.tile([128, S_HALF], F32, tag="rec")
                            osb = outs.tile([128, S_HALF], BF16, tag="osb")
                            step = 512
                            # sums pass 1: tiles 0..14 (ready at exp14)
                            for c in range(0, S_HALF, step):
                                nc.tensor.matmul(
                                    sacc[:, c : c + step],
                                    ones_b[:],
                                    eacc[:, c : c + step],
                                    start=True,
                                    stop=False,
                                )
                            # sums pass 2 + normalize, chunked; the FINAL
                            # tail has no successor to hide behind, so it
                            # uses 256-col chunks and 4 parallel output
                            # DMAs (one 512KB store on a single ~22GB/s
                            # queue would add ~6us to the wall clock)
                            for c in range(0, S_HALF, step):
                                nc.tensor.matmul(
                                    sacc[:, c : c + step],
                                    ones_b[:],
                                    ets[T_TILES - 1][:, c : c + step],
                                    start=False,
                                    stop=True,
                                )
                                nc.vector.reciprocal_approx_fast(
                                    out=rec[:, c : c + step],
                                    in_=sacc[:, c : c + step],
                                )
                                nc.vector.tensor_mul(
                                    osb[:, c : c + step],
                                    oacc[:, c : c + step],
                                    rec[:, c : c + step],
                                )
                                # DMA split finer than the compute chunks
                                # on the final tail: its stores can't hide
                                # behind a successor, and one queue only
                                # moves ~22GB/s.
                                dstep = 256 if final else step
                                for dc in range(c, c + step, dstep):
                                    nc.sync.dma_start(
                                        out=ot[i, :, s0 + dc : s0 + dc + dstep],
                                        in_=osb[:, dc : dc + dstep],
                                    )
                        return tail

                    pending_tail[0] = make_tail()

            flush_tail(final=True)

    nc.finalize()
    return nc


def _get_program():
    if "nc" not in _cache:
        _cache["nc"] = _build_program()
    return _cache["nc"]


def kernel(q: np.ndarray, k: np.ndarray, v: np.ndarray) -> np.ndarray:
    import ml_dtypes
    from concourse.bass_utils import run_bass_kernel_spmd

    nc = _get_program()

    bf16 = ml_dtypes.bfloat16
    q4 = np.asarray(q, dtype=np.float32).reshape(BH, S, D)
    k4 = np.asarray(k, dtype=np.float32).reshape(BH, S, D)
    v4 = np.asarray(v, dtype=np.float32).reshape(BH, S, D)

    in_maps = []
    for core in range(N_CORES):
        sl = slice(core * BH_PER_CORE, (core + 1) * BH_PER_CORE)
        in_maps.append(
            {
                "qt": np.ascontiguousarray(
                    q4[sl].transpose(0, 2, 1).astype(bf16)
                ),
                "kt": np.ascontiguousarray(
                    k4[sl].transpose(0, 2, 1).astype(bf16)
                ),
                # [i, t*128+p, d] -> [i, p, t*128+d]
                "v": np.ascontiguousarray(
                    v4[sl]
                    .reshape(BH_PER_CORE, T_TILES, 128, D)
                    .transpose(0, 2, 1, 3)
                    .reshape(BH_PER_CORE, 128, T_TILES * D)
                    .astype(bf16)
                ),
            }
        )

    res = run_bass_kernel_spmd(nc, in_maps, core_ids=list(range(N_CORES)))

    out = np.empty((BH, S, D), dtype=np.float32)
    for core in range(N_CORES):
        ot = res.results[core]["ot"]  # [BH_PER_CORE, D, S] bf16
        out[core * BH_PER_CORE : (core + 1) * BH_PER_CORE] = (
            ot.transpose(0, 2, 1).astype(np.float32)
        )
    return out.reshape(B, H, S, D)


# revision 39
# speedup vs baseline: 1.2043x; 1.2043x over previous
"""Trainium2 Bass kernel for multi-head attention.

Problem: B=4, H=16, S=2048, D=128, fp32.
  scores = (q @ k^T) / sqrt(128); probs = softmax(scores, -1); out = probs @ v

Sharding: 64 (b,h) pairs -> 8 cores x 8 pairs. Fully independent per pair.

Layout (all-bf16 datapath; T-layout per (b,h) pair; s in halves of 1024):
  qT, kT: [D=128, S=2048] bf16 in SBUF. For each t-tile (128 keys):
    scoresT[t, s] = kT[:, t-tile].T @ qT   (PE, 2 matmuls N=512, bf16)
    expT = exp(scoresT / sqrt(D)) -> bf16  (ACT, fused scale, psum->sbuf)
    outT[d, s] += v_tile.T @ expT          (PE, 2 matmuls N=512, accum)
    eacc += expT                           (DVE bf16 adds, 2x mode)
  Denominator: eacc (tiles 0..14) collapsed over the 128 t-partitions by a
  PE ones-matmul pass started during exp15, tile 15 folded by a second
  accumulating ones-matmul pass; reciprocal (DVE approx, chunked),
  out = outT * rec (DVE), DMA out bf16 (host upcasts to fp32).

  The whole tail of half h (PV of tiles 14..15, sums, normalize, store) is
  EMITTED inside half h+1's t-loop after its second exp: the PE executes
  its queue in order, so this puts the next half's first QKs ahead of the
  exp15-gated tail matmuls -- ACT never idles at half boundaries (this
  was a ~1.5us stall per half when the tail was emitted inline). The PV
  consume lag is 2 tiles to match.

PSUM: 3 score slots x 2 banks (one doubles as the sums buffer during the
deferred tail) + outT accumulator 2 banks = 8 banks.

bf16 end-to-end error vs the fp32 reference is ~3.5e-3 relative
(threshold 2e-2).
"""

import sys

sys.path.insert(0, "/opt/trn_rl_repo")

import numpy as np

B, H, S, D = 4, 16, 2048, 128
N_CORES = 8
BH = B * H                      # 64 pairs
BH_PER_CORE = BH // N_CORES     # 8
T_TILES = S // 128              # 16
S_HALF = S // 2                 # 1024
SCALE = float(D) ** -0.5
T_DVE = 8                       # this tile's exp runs on DVE (Schraudolph)

_cache = {}


def _build_program():
    import concourse.tile as tile
    from concourse import bacc, mybir

    F32 = mybir.dt.float32
    BF16 = mybir.dt.bfloat16
    I32 = mybir.dt.int32
    # Schraudolph: exp(s*SCALE) ~= bitcast(int32(A*s + B)), rms err ~1.8%;
    # errors mostly cancel between softmax numerator and denominator
    A_S = (2.0 ** 23) / 0.6931471805599453 * SCALE
    B_S = 127.0 * 2.0 ** 23 - 486411.0

    nc = bacc.Bacc("TRN2", target_bir_lowering=False, debug=False)

    qt = nc.dram_tensor("qt", [BH_PER_CORE, D, S], BF16, kind="ExternalInput")
    kt = nc.dram_tensor("kt", [BH_PER_CORE, D, S], BF16, kind="ExternalInput")
    # v pre-shuffled on host to [p, t, d] so the load is fully contiguous
    v = nc.dram_tensor("v", [BH_PER_CORE, 128, T_TILES * D], BF16, kind="ExternalInput")
    ot = nc.dram_tensor("ot", [BH_PER_CORE, D, S], BF16, kind="ExternalOutput")

    with tile.TileContext(nc) as tc:
        with (
            tc.tile_pool(name="const", bufs=1) as const,
            tc.tile_pool(name="rin", bufs=2) as rin,
            tc.tile_pool(name="exps", bufs=12) as exps,
            tc.tile_pool(name="accp", bufs=2) as accp,
            tc.tile_pool(name="outs", bufs=6) as outs,
            tc.tile_pool(name="sch", bufs=2) as sch,
            tc.tile_pool(name="psc", bufs=3, space="PSUM") as psc,
            tc.tile_pool(name="pacc", bufs=1, space="PSUM") as pacc,
        ):
            ones_f = const.tile([128, 128], F32)
            nc.vector.memset(ones_f[:], 1.0)
            ones_b = const.tile([128, 128], BF16)
            nc.vector.tensor_copy(ones_b[:], ones_f[:])

            pending_tail = [None]

            def flush_tail(final=False):
                if pending_tail[0] is not None:
                    t_fn = pending_tail[0]
                    pending_tail[0] = None
                    t_fn(final)

            for i in range(BH_PER_CORE):
                q_r = rin.tile([D, S], BF16, tag="q_r")
                k_r = rin.tile([D, S], BF16, tag="k_r")
                v_r = rin.tile([128, T_TILES, D], BF16, tag="v_r")
                # order: what the first tiles need comes first; v split so
                # the first PV tiles don't wait on one huge transfer
                nc.sync.dma_start(out=k_r[:, :128], in_=kt[i, :, :128])
                nc.sync.dma_start(out=q_r[:, :512], in_=qt[i, :, :512])
                vv = v[i].rearrange("p (t d) -> p t d", t=T_TILES)
                nc.sync.dma_start(out=v_r[:, 0:2], in_=vv[:, 0:2])
                nc.sync.dma_start(out=k_r[:, 128:512], in_=kt[i, :, 128:512])
                nc.sync.dma_start(out=q_r[:, 512:S_HALF], in_=qt[i, :, 512:S_HALF])
                nc.sync.dma_start(out=v_r[:, 2:4], in_=vv[:, 2:4])
                nc.sync.dma_start(out=k_r[:, 512:S_HALF], in_=kt[i, :, 512:S_HALF])
                nc.sync.dma_start(out=v_r[:, 4:8], in_=vv[:, 4:8])
                nc.sync.dma_start(out=v_r[:, 8:12], in_=vv[:, 8:12])
                nc.sync.dma_start(out=v_r[:, 12:16], in_=vv[:, 12:16])
                nc.sync.dma_start(out=q_r[:, S_HALF:], in_=qt[i, :, S_HALF:])
                nc.sync.dma_start(out=k_r[:, S_HALF:], in_=kt[i, :, S_HALF:])

                for h in range(2):
                    s0 = h * S_HALF
                    oacc = pacc.tile([128, S_HALF], F32, tag="oacc")
                    eacc = accp.tile([128, S_HALF], BF16, tag="eacc")

                    ets = [None] * T_TILES

                    def pv(t, oacc=oacc, v_r=v_r, ets=ets):
                        for c in range(0, S_HALF, 512):
                            nc.tensor.matmul(
                                oacc[:, c : c + 512],
                                v_r[:, t, :],
                                ets[t][:, c : c + 512],
                                start=(t == 0),
                                stop=(t == T_TILES - 1),
                            )

                    def consume(t, eacc=eacc, ets=ets, pv=pv):
                        pv(t)
                        if t == 1:
                            nc.vector.tensor_add(eacc[:], ets[0][:], ets[1][:])
                        elif 1 < t < T_TILES - 1:
                            nc.vector.tensor_add(eacc[:], eacc[:], ets[t][:])

                    for t in range(T_TILES):
                        sc = psc.tile([128, S_HALF], F32, tag="sc")
                        for c in range(0, S_HALF, 512):
                            nc.tensor.matmul(
                                sc[:, c : c + 512],
                                k_r[:, t * 128 : (t + 1) * 128],
                                q_r[:, s0 + c : s0 + c + 512],
                                start=True,
                                stop=True,
                            )
                        ets[t] = exps.tile(
                            [128, S_HALF], BF16, tag="et", name=f"et_{t}"
                        )
                        if t == T_DVE:
                            # exp on DVE (Schraudolph), mid-half where ACT
                            # has backlog to cover the skipped slot. The
                            # int32 bits are re-read as fp32 and rounded
                            # to bf16 (f32r matmuls reject un-rounded
                            # bitcast inputs).
                            tmp = sch.tile(
                                [128, S_HALF], I32, tag="ts_tmp"
                            )
                            nc.vector.tensor_scalar(
                                tmp[:],
                                sc[:],
                                A_S,
                                B_S,
                                mybir.AluOpType.mult,
                                mybir.AluOpType.add,
                            )
                            nc.vector.tensor_copy(
                                ets[t][:], tmp[:].bitcast(F32)
                            )
                        else:
                            nc.scalar.activation(
                                ets[t][:],
                                sc[:],
                                mybir.ActivationFunctionType.Exp,
                                scale=SCALE,
                            )
                        if t == 1:
                            # previous half's tail lands here: after this
                            # half's first two QKs in PE program order
                            flush_tail()
                        if t >= 2:
                            consume(t - 2)

                    def make_tail(i=i, s0=s0, oacc=oacc, eacc=eacc,
                                  ets=ets, pv=pv, consume=consume):
                        def tail(final):
                            consume(T_TILES - 2)   # PV14 + add14
                            pv(T_TILES - 1)        # PV15
                            sacc = psc.tile(
                                [128, S_HALF], F32, tag="sc", name="sacc"
                            )
                            rec = outs.tile([128, S_HALF], F32, tag="rec")
                            osb = outs.tile([128, S_HALF], BF16, tag="osb")
                            step = 512
                            # sums pass 1: tiles 0..14 (ready at exp14)
                            for c in range(0, S_HALF, step):
                                nc.tensor.matmul(
                                    sacc[:, c : c + step],
                                    ones_b[:],
                                    eacc[:, c : c + step],
                                    start=True,
                                    stop=False,
                                )
                            # sums pass 2 + normalize, chunked; the FINAL
                            # tail has no successor to hide behind, so it
                            # uses 256-col chunks and 4 parallel output
                            # DMAs (one 512KB store on a single ~22GB/s
                            # queue would add ~6us to the wall clock)
                            for c in range(0, S_HALF, step):
                                nc.tensor.matmul(
                                    sacc[:, c : c + step],
                                    ones_b[:],
                                    ets[T_TILES - 1][:, c : c + step],
                                    start=False,
                                    stop=True,
                                )
                                nc.vector.reciprocal_approx_fast(
                                    out=rec[:, c : c + step],
                                    in_=sacc[:, c : c + step],
                                )
                                nc.vector.tensor_mul(
                                    osb[:, c : c + step],
                                    oacc[:, c : c + step],
                                    rec[:, c : c + step],
                                )
                                nc.sync.dma_start(
                                    out=ot[i, :, s0 + c : s0 + c + step],
                                    in_=osb[:, c : c + step],
                                )
                        return tail

                    pending_tail[0] = make_tail()

            flush_tail(final=True)

    nc.finalize()
    return nc


def _get_program():
    if "nc" not in _cache:
        _cache["nc"] = _build_program()
    return _cache["nc"]


def kernel(q: np.ndarray, k: np.ndarray, v: np.ndarray) -> np.ndarray:
    import ml_dtypes
    from concourse.bass_utils import run_bass_kernel_spmd

    nc = _get_program()

    bf16 = ml_dtypes.bfloat16
    q4 = np.asarray(q, dtype=np.float32).reshape(BH, S, D)
    k4 = np.asarray(k, dtype=np.float32).reshape(BH, S, D)
    v4 = np.asarray(v, dtype=np.float32).reshape(BH, S, D)

    in_maps = []
    for core in range(N_CORES):
        sl = slice(core * BH_PER_CORE, (core + 1) * BH_PER_CORE)
        in_maps.append(
            {
                "qt": np.ascontiguousarray(
                    q4[sl].transpose(0, 2, 1).astype(bf16)
                ),
                "kt": np.ascontiguousarray(
                    k4[sl].transpose(0, 2, 1).astype(bf16)
                ),
                # [i, t*128+p, d] -> [i, p, t*128+d]
                "v": np.ascontiguousarray(
                    v4[sl]
                    .reshape(BH_PER_CORE, T_TILES, 128, D)
                    .transpose(0, 2, 1, 3)
                    .reshape(BH_PER_CORE, 128, T_TILES * D)
                    .astype(bf16)
                ),
            }
        )

    res = run_bass_kernel_spmd(nc, in_maps, core_ids=list(range(N_CORES)))

    out = np.empty((BH, S, D), dtype=np.float32)
    for core in range(N_CORES):
        ot = res.results[core]["ot"]  # [BH_PER_CORE, D, S] bf16
        out[core * BH_PER_CORE : (core + 1) * BH_PER_CORE] = (
            ot.transpose(0, 2, 1).astype(np.float32)
        )
    return out.reshape(B, H, S, D)


# revision 40
# speedup vs baseline: 1.2117x; 1.0062x over previous
"""Trainium2 Bass kernel for multi-head attention.

Problem: B=4, H=16, S=2048, D=128, fp32.
  scores = (q @ k^T) / sqrt(128); probs = softmax(scores, -1); out = probs @ v

Sharding: 64 (b,h) pairs -> 8 cores x 8 pairs. Fully independent per pair.

Layout (all-bf16 datapath; T-layout per (b,h) pair; s in halves of 1024):
  qT, kT: [D=128, S=2048] bf16 in SBUF. For each t-tile (128 keys):
    scoresT[t, s] = kT[:, t-tile].T @ qT   (PE, 2 matmuls N=512, bf16)
    expT = exp(scoresT / sqrt(D)) -> bf16  (ACT, fused scale, psum->sbuf)
    outT[d, s] += v_tile.T @ expT          (PE, 2 matmuls N=512, accum)
    eacc += expT                           (DVE bf16 adds, 2x mode)
  Denominator: eacc (tiles 0..14) collapsed over the 128 t-partitions by a
  PE ones-matmul pass started during exp15, tile 15 folded by a second
  accumulating ones-matmul pass; reciprocal (DVE approx, chunked),
  out = outT * rec (DVE), DMA out bf16 (host upcasts to fp32).

  The whole tail of half h (PV of tiles 14..15, sums, normalize, store) is
  EMITTED inside half h+1's t-loop after its second exp: the PE executes
  its queue in order, so this puts the next half's first QKs ahead of the
  exp15-gated tail matmuls -- ACT never idles at half boundaries (this
  was a ~1.5us stall per half when the tail was emitted inline). The PV
  consume lag is 2 tiles to match.

PSUM: 3 score slots x 2 banks (one doubles as the sums buffer during the
deferred tail) + outT accumulator 2 banks = 8 banks.

bf16 end-to-end error vs the fp32 reference is ~3.5e-3 relative
(threshold 2e-2).
"""

import sys

sys.path.insert(0, "/opt/trn_rl_repo")

import numpy as np

B, H, S, D = 4, 16, 2048, 128
N_CORES = 8
BH = B * H                      # 64 pairs
BH_PER_CORE = BH // N_CORES     # 8
T_TILES = S // 128              # 16
S_HALF = S // 2                 # 1024
SCALE = float(D) ** -0.5
T_DVE = 8                       # this tile's exp runs on DVE (Schraudolph)

_cache = {}


def _build_program():
    import concourse.tile as tile
    from concourse import bacc, mybir

    F32 = mybir.dt.float32
    BF16 = mybir.dt.bfloat16
    I32 = mybir.dt.int32
    # Schraudolph: exp(s*SCALE) ~= bitcast(int32(A*s + B)), rms err ~1.8%;
    # errors mostly cancel between softmax numerator and denominator
    A_S = (2.0 ** 23) / 0.6931471805599453 * SCALE
    B_S = 127.0 * 2.0 ** 23 - 486411.0

    nc = bacc.Bacc("TRN2", target_bir_lowering=False, debug=False)

    qt = nc.dram_tensor("qt", [BH_PER_CORE, D, S], BF16, kind="ExternalInput")
    kt = nc.dram_tensor("kt", [BH_PER_CORE, D, S], BF16, kind="ExternalInput")
    # v pre-shuffled on host to [p, t, d] so the load is fully contiguous
    v = nc.dram_tensor("v", [BH_PER_CORE, 128, T_TILES * D], BF16, kind="ExternalInput")
    ot = nc.dram_tensor("ot", [BH_PER_CORE, D, S], BF16, kind="ExternalOutput")

    with tile.TileContext(nc) as tc:
        with (
            tc.tile_pool(name="const", bufs=1) as const,
            tc.tile_pool(name="rin", bufs=2) as rin,
            tc.tile_pool(name="exps", bufs=12) as exps,
            tc.tile_pool(name="accp", bufs=2) as accp,
            tc.tile_pool(name="outs", bufs=6) as outs,
            tc.tile_pool(name="sch", bufs=2) as sch,
            tc.tile_pool(name="psc", bufs=3, space="PSUM") as psc,
            tc.tile_pool(name="pacc", bufs=1, space="PSUM") as pacc,
        ):
            ones_f = const.tile([128, 128], F32)
            nc.vector.memset(ones_f[:], 1.0)
            ones_b = const.tile([128, 128], BF16)
            nc.vector.tensor_copy(ones_b[:], ones_f[:])

            pending_tail = [None]

            def flush_tail(final=False):
                if pending_tail[0] is not None:
                    t_fn = pending_tail[0]
                    pending_tail[0] = None
                    t_fn(final)

            for i in range(BH_PER_CORE):
                q_r = rin.tile([D, S], BF16, tag="q_r")
                k_r = rin.tile([D, S], BF16, tag="k_r")
                v_r = rin.tile([128, T_TILES, D], BF16, tag="v_r")
                # order: what the first tiles need comes first; v split so
                # the first PV tiles don't wait on one huge transfer
                nc.sync.dma_start(out=k_r[:, :128], in_=kt[i, :, :128])
                nc.sync.dma_start(out=q_r[:, :512], in_=qt[i, :, :512])
                vv = v[i].rearrange("p (t d) -> p t d", t=T_TILES)
                nc.sync.dma_start(out=v_r[:, 0:2], in_=vv[:, 0:2])
                nc.sync.dma_start(out=k_r[:, 128:512], in_=kt[i, :, 128:512])
                nc.sync.dma_start(out=q_r[:, 512:S_HALF], in_=qt[i, :, 512:S_HALF])
                nc.sync.dma_start(out=v_r[:, 2:4], in_=vv[:, 2:4])
                nc.sync.dma_start(out=k_r[:, 512:S_HALF], in_=kt[i, :, 512:S_HALF])
                nc.sync.dma_start(out=v_r[:, 4:8], in_=vv[:, 4:8])
                nc.sync.dma_start(out=v_r[:, 8:12], in_=vv[:, 8:12])
                nc.sync.dma_start(out=v_r[:, 12:16], in_=vv[:, 12:16])
                nc.sync.dma_start(out=q_r[:, S_HALF:], in_=qt[i, :, S_HALF:])
                nc.sync.dma_start(out=k_r[:, S_HALF:], in_=kt[i, :, S_HALF:])

                for h in range(2):
                    s0 = h * S_HALF
                    oacc = pacc.tile([128, S_HALF], F32, tag="oacc")
                    eacc = accp.tile([128, S_HALF], BF16, tag="eacc")

                    ets = [None] * T_TILES

                    def pv(t, oacc=oacc, v_r=v_r, ets=ets):
                        for c in range(0, S_HALF, 512):
                            nc.tensor.matmul(
                                oacc[:, c : c + 512],
                                v_r[:, t, :],
                                ets[t][:, c : c + 512],
                                start=(t == 0),
                                stop=(t == T_TILES - 1),
                            )

                    def consume(t, eacc=eacc, ets=ets, pv=pv):
                        pv(t)
                        if t == 1:
                            nc.vector.tensor_add(eacc[:], ets[0][:], ets[1][:])
                        elif 1 < t < T_TILES - 1:
                            nc.vector.tensor_add(eacc[:], eacc[:], ets[t][:])

                    for t in range(T_TILES):
                        sc = psc.tile([128, S_HALF], F32, tag="sc")
                        for c in range(0, S_HALF, 512):
                            nc.tensor.matmul(
                                sc[:, c : c + 512],
                                k_r[:, t * 128 : (t + 1) * 128],
                                q_r[:, s0 + c : s0 + c + 512],
                                start=True,
                                stop=True,
                            )
                        ets[t] = exps.tile(
                            [128, S_HALF], BF16, tag="et", name=f"et_{t}"
                        )
                        if t == T_DVE:
                            # exp on DVE (Schraudolph), mid-half where ACT
                            # has backlog to cover the skipped slot. The
                            # int32 bits are re-read as fp32 and rounded
                            # to bf16 (f32r matmuls reject un-rounded
                            # bitcast inputs).
                            tmp = sch.tile(
                                [128, S_HALF], I32, tag="ts_tmp"
                            )
                            nc.vector.tensor_scalar(
                                tmp[:],
                                sc[:],
                                A_S,
                                B_S,
                                mybir.AluOpType.mult,
                                mybir.AluOpType.add,
                            )
                            nc.vector.tensor_copy(
                                ets[t][:], tmp[:].bitcast(F32)
                            )
                        else:
                            nc.scalar.activation(
                                ets[t][:],
                                sc[:],
                                mybir.ActivationFunctionType.Exp,
                                scale=SCALE,
                            )
                        if t == 1:
                            # previous half's tail lands here: after this
                            # half's first two QKs in PE program order
                            flush_tail()
                        if t >= 2:
                            consume(t - 2)

                    def make_tail(i=i, s0=s0, oacc=oacc, eacc=eacc,
                                  ets=ets, pv=pv, consume=consume):
                        def tail(final):
                            consume(T_TILES - 2)   # PV14 + add14
                            pv(T_TILES - 1)        # PV15
                            sacc = psc.tile(
                                [128, S_HALF], F32, tag="sc", name="sacc"
                            )
                            rec = outs.tile([128, S_HALF], F32, tag="rec")
                            osb = outs.tile([128, S_HALF], BF16, tag="osb")
                            step = 512
                            # fold tile 15 on DVE, then a SINGLE ones-pass
                            # (ACT now has slack to absorb the slightly
                            # longer add chain; saves 2 PE matmuls/half)
                            for c in range(0, S_HALF, step):
                                nc.vector.tensor_add(
                                    eacc[:, c : c + step],
                                    eacc[:, c : c + step],
                                    ets[T_TILES - 1][:, c : c + step],
                                )
                                nc.tensor.matmul(
                                    sacc[:, c : c + step],
                                    ones_b[:],
                                    eacc[:, c : c + step],
                                    start=True,
                                    stop=True,
                                )
                                nc.vector.reciprocal_approx_fast(
                                    out=rec[:, c : c + step],
                                    in_=sacc[:, c : c + step],
                                )
                                nc.vector.tensor_mul(
                                    osb[:, c : c + step],
                                    oacc[:, c : c + step],
                                    rec[:, c : c + step],
                                )
                                nc.sync.dma_start(
                                    out=ot[i, :, s0 + c : s0 + c + step],
                                    in_=osb[:, c : c + step],
                                )
                        return tail

                    pending_tail[0] = make_tail()

            flush_tail(final=True)

    nc.finalize()
    return nc


def _get_program():
    if "nc" not in _cache:
        _cache["nc"] = _build_program()
    return _cache["nc"]


def kernel(q: np.ndarray, k: np.ndarray, v: np.ndarray) -> np.ndarray:
    import ml_dtypes
    from concourse.bass_utils import run_bass_kernel_spmd

    nc = _get_program()

    bf16 = ml_dtypes.bfloat16
    q4 = np.asarray(q, dtype=np.float32).reshape(BH, S, D)
    k4 = np.asarray(k, dtype=np.float32).reshape(BH, S, D)
    v4 = np.asarray(v, dtype=np.float32).reshape(BH, S, D)

    in_maps = []
    for core in range(N_CORES):
        sl = slice(core * BH_PER_CORE, (core + 1) * BH_PER_CORE)
        in_maps.append(
            {
                "qt": np.ascontiguousarray(
                    q4[sl].transpose(0, 2, 1).astype(bf16)
                ),
                "kt": np.ascontiguousarray(
                    k4[sl].transpose(0, 2, 1).astype(bf16)
                ),
                # [i, t*128+p, d] -> [i, p, t*128+d]
                "v": np.ascontiguousarray(
                    v4[sl]
                    .reshape(BH_PER_CORE, T_TILES, 128, D)
                    .transpose(0, 2, 1, 3)
                    .reshape(BH_PER_CORE, 128, T_TILES * D)
                    .astype(bf16)
                ),
            }
        )

    res = run_bass_kernel_spmd(nc, in_maps, core_ids=list(range(N_CORES)))

    out = np.empty((BH, S, D), dtype=np.float32)
    for core in range(N_CORES):
        ot = res.results[core]["ot"]  # [BH_PER_CORE, D, S] bf16
        out[core * BH_PER_CORE : (core + 1) * BH_PER_CORE] = (
            ot.transpose(0, 2, 1).astype(np.float32)
        )
    return out.reshape(B, H, S, D)
